# revision 1
# baseline (speedup 1.0000x reference)
"""DOM pooling (segment mean+max over pulses, then linear projection) on 8 trn2 cores.

Strategy:
  Host: bucket DOMs by exact pulse count k ("classes"); deal DOMs of each class
  round-robin across the 8 cores so every core has an identical class structure
  (same per-class DOM count m_k, padded with dummy DOMs). Full 128-DOM windows
  are emitted per class; the leftover (<128) DOMs of every class are packed
  together into shared "ragged" windows (sorted by k, per-DOM slots padded to
  the window capacity by duplicating the DOM's first slot — max-neutral; the
  sum is corrected on device by subtracting padcnt*x0 and scaled by 1/k).
  Each core gets a gathered slot buffer where a DOM's k pulse embeddings are
  stored embed-major (64 x k contiguous), so device reads are sequential.

  Device (one NEFF, SPMD on 8 cores), per 128-DOM window:
    - batched contiguous DMA loads
    - DVE reduce_sum / reduce_max over the slots (contiguous inner axis)
    - PE transpose of [sum|max] concat (128x128) -> PSUM (feat-major)
    - projection matmul out^T = Wk^T.T @ concatT (mean scaling 1/k folded into
      the sum-feature rows of the per-class weights; ragged windows scale on
      DVE and use unscaled weights)
    - ACT adds bias during PSUM->SBUF copy; batched DMA writes out^T.

  Host: scatter per-core transposed outputs back to the full (num_doms, 64).
"""
import sys

import numpy as np

for _p in ("/opt/trn_rl_repo",):
    if _p not in sys.path:
        sys.path.append(_p)

from concourse import bacc
import concourse.mybir as mybir
import concourse.tile as tile
from concourse.bass_utils import run_bass_kernel_spmd
from concourse.masks import make_identity

NCORES = 8
D = 64
FP32 = mybir.dt.float32

last_exec_ns = None  # set when KERNEL_TRACE=1


def _plan(counts):
    """Class/window structure shared by all cores (derived from global counts).

    Returns:
      full_cls: list of (k, fw, col0, base) classes with fw full windows
      rag_cls:  list of (k, r) leftover doms per class (class order)
      rag_win:  list of (k_w, base, col0) ragged windows
      ndcp:     output columns per core
      s_elems:  slot buffer elements per core
    """
    kmax = int(counts.max()) if counts.size else 0
    n_k = np.bincount(counts, minlength=kmax + 1)
    full_cls = []
    rag_cls = []
    col = 0
    slot = 0
    for k in range(1, kmax + 1):
        if n_k[k] == 0:
            continue
        m = -(-int(n_k[k]) // NCORES)
        fw = m // 128
        r = m % 128
        if fw:
            full_cls.append((k, fw, col, slot))
            col += fw * 128
            slot += fw * 128 * k * D
        if r:
            rag_cls.append((k, r))
    # ragged windows: doms in class order (ascending k); capacity = max k in win
    rag_win = []
    R = sum(r for _, r in rag_cls)
    if R:
        ks = np.concatenate([np.full(r, k, np.int32) for k, r in rag_cls])
        RW = -(-R // 128)
        for j in range(RW):
            seg = ks[j * 128 : (j + 1) * 128]
            kw = int(seg.max())
            rag_win.append((kw, slot, col + j * 128))
            slot += 128 * kw * D
        col += RW * 128
    return full_cls, rag_cls, rag_win, col, slot


def _build_nc(full_cls, rag_win, ndcp, s_elems, nwt, nrw):
    nc = bacc.Bacc(None)
    slots_t = nc.dram_tensor("slots", [s_elems], FP32, kind="ExternalInput")
    wts_t = nc.dram_tensor("wts", [nwt * 128, D], FP32, kind="ExternalInput")
    b_t = nc.dram_tensor("b", [D, 1], FP32, kind="ExternalInput")
    if nrw:
        rt_t = nc.dram_tensor("rt", [nrw * 128, 2], FP32, kind="ExternalInput")
    out_t = nc.dram_tensor("out", [D, ndcp], FP32, kind="ExternalOutput")

    with tile.TileContext(nc) as tc:
        with (
            tc.tile_pool(name="const", bufs=1) as constp,
            tc.tile_pool(name="inp", bufs=6) as inp,
            tc.tile_pool(name="mid", bufs=4) as midp,
            tc.tile_pool(name="outp", bufs=4) as outp,
            tc.tile_pool(name="psA", bufs=4, space="PSUM") as psA,
            tc.tile_pool(name="psB", bufs=4, space="PSUM") as psB,
        ):
            ident = constp.tile([128, 128], FP32)
            make_identity(nc, ident[:])
            wt_sb = constp.tile([128, nwt * D], FP32)
            nc.sync.dma_start(
                wt_sb[:].rearrange("p (j e) -> p j e", e=D),
                wts_t[:, :].rearrange("(j p) e -> p j e", p=128),
            )
            b_sb = constp.tile([D, 1], FP32)
            nc.sync.dma_start(b_sb[:], b_t[:])
            if nrw:
                rt_sb = constp.tile([128, nrw * 2], FP32)
                nc.sync.dma_start(
                    rt_sb[:].rearrange("p (j c) -> p j c", c=2),
                    rt_t[:, :].rearrange("(j p) c -> p j c", p=128),
                )

            def window_tail(cat, out_ap, p, jwt):
                """cat: (p, 128) [sum|max] slice; out_ap: (D, p) slice."""
                catT_ps = psA.tile([128, 128], FP32, space="PSUM", tag="ps")
                nc.tensor.transpose(
                    out=catT_ps[:, :p], in_=cat, identity=ident[:p, :p]
                )
                catT = midp.tile([128, 128], FP32, tag="catT")
                nc.scalar.copy(catT[:, :p], catT_ps[:, :p])
                proj_ps = psB.tile([D, 128], FP32, space="PSUM", tag="proj")
                nc.tensor.matmul(
                    proj_ps[:, :p],
                    lhsT=wt_sb[:, jwt * D : (jwt + 1) * D],
                    rhs=catT[:, :p],
                    start=True,
                    stop=True,
                )
                nc.scalar.activation(
                    out_ap, proj_ps[:, :p],
                    mybir.ActivationFunctionType.Identity, bias=b_sb[:, :1],
                )

            # ---- full per-class windows --------------------------------
            for jcls, (k, fw, col0, base) in enumerate(full_cls):
                G = max(1, min(8, 64 // k))
                g = 0
                while g < fw:
                    gw = min(G, fw - g)
                    in_t = inp.tile([128, gw * k * D], FP32, tag="in")
                    src = slots_t[
                        base + g * 128 * k * D : base + (g + gw) * 128 * k * D
                    ].rearrange("(w d f) -> d w f", w=gw, d=128)
                    nc.sync.dma_start(
                        in_t[:].rearrange("d (w f) -> d w f", w=gw), src
                    )
                    cat_g = midp.tile([128, 8 * 128], FP32, tag="cat")
                    co = cat_g[:, : gw * 128].rearrange("d (w c) -> d w c", c=128)
                    if k == 1:
                        v = in_t[:].rearrange("d (w e) -> d w e", w=gw)
                        nc.vector.tensor_copy(co[:, :, 0:D], v)
                        nc.vector.tensor_copy(co[:, :, D:128], v)
                    else:
                        view = in_t[:].rearrange("d (w e s) -> d w e s", w=gw, s=k)
                        nc.vector.reduce_sum(co[:, :, 0:D], view, axis=mybir.AxisListType.X)
                        nc.vector.reduce_max(co[:, :, D:128], view, axis=mybir.AxisListType.X)
                    out_sb = outp.tile([64, 8 * 128], FP32, tag="out")
                    for w in range(gw):
                        window_tail(
                            cat_g[:, w * 128 : (w + 1) * 128],
                            out_sb[:, w * 128 : (w + 1) * 128],
                            128, jcls,
                        )
                    nc.sync.dma_start(
                        out_t[:, col0 + g * 128 : col0 + (g + gw) * 128],
                        out_sb[:, : gw * 128],
                    )
                    g += gw

            # ---- ragged windows (mixed k, capacity k_w) ----------------
            juns = len(full_cls)  # unscaled weight block index
            for j, (kw, base, col0) in enumerate(rag_win):
                in_t = inp.tile([128, kw * D], FP32, tag="in")
                nc.sync.dma_start(
                    in_t[:], slots_t[base : base + 128 * kw * D].rearrange(
                        "(d f) -> d f", d=128
                    ),
                )
                cat_g = midp.tile([128, 8 * 128], FP32, tag="cat")
                view = in_t[:].rearrange("d (e s) -> d e s", s=kw)
                sraw = midp.tile([128, D], FP32, tag="sraw")
                nc.vector.reduce_sum(sraw[:], view, axis=mybir.AxisListType.X)
                nc.vector.reduce_max(cat_g[:, D:128], view, axis=mybir.AxisListType.X)
                # sum correction: (sraw - padcnt*x0) * recip_k
                x0 = view[:, :, 0]
                tmp = midp.tile([128, D], FP32, tag="tmp")
                nc.vector.tensor_scalar_mul(
                    tmp[:], x0, rt_sb[:, j * 2 + 1 : j * 2 + 2]
                )
                nc.vector.tensor_tensor(
                    out=tmp[:], in0=sraw[:], in1=tmp[:], op=mybir.AluOpType.subtract
                )
                nc.vector.tensor_scalar_mul(
                    cat_g[:, 0:D], tmp[:], rt_sb[:, j * 2 : j * 2 + 1]
                )
                out_sb = outp.tile([64, 8 * 128], FP32, tag="out")
                window_tail(cat_g[:, 0:128], out_sb[:, :128], 128, juns)
                nc.sync.dma_start(out_t[:, col0 : col0 + 128], out_sb[:, :128])
    nc.finalize()
    return nc


def kernel(pulse_embeddings, pulse_to_dom_idx, num_doms, proj_w, proj_b):
    global last_exec_ns
    import os

    E = np.ascontiguousarray(np.asarray(pulse_embeddings, dtype=np.float32))
    idx = np.asarray(pulse_to_dom_idx).astype(np.int64)
    nd = int(num_doms)
    W = np.asarray(proj_w, dtype=np.float32)   # (D, 2D)
    b = np.asarray(proj_b, dtype=np.float32)   # (D,)

    counts = np.bincount(idx, minlength=nd)
    full_cls, rag_cls, rag_win, ndcp, s_elems = _plan(counts)
    nwt = len(full_cls) + 1
    nrw = len(rag_win)

    # ---- host-side dom assignment --------------------------------------
    dom_order = np.argsort(counts, kind="stable")
    cs = counts[dom_order]
    n0 = int((counts == 0).sum())
    dom_core = np.full(nd, -1, np.int32)
    dom_col = np.full(nd, -1, np.int32)

    # per-class bookkeeping (shared across cores)
    kmax = int(counts.max()) if counts.size else 0
    n_k = np.bincount(counts, minlength=kmax + 1)
    full_map = {k: (jc, fw, col0, base) for jc, (k, fw, col0, base) in enumerate(full_cls)}
    # ragged: position of each class's leftover run inside the ragged region
    rag_off = {}
    ro = 0
    for k, r in rag_cls:
        rag_off[k] = ro
        ro += r
    R = ro
    rag_col0 = rag_win[0][2] - 0 if rag_win else ndcp  # col of ragged dom 0
    if rag_win:
        rag_col0 = rag_win[0][2]

    off = n0
    # per (class-k, core): number of real doms; and split into full/ragged
    cls_meta = []  # (k, m, n_real, fw, r)
    for k in range(1, kmax + 1):
        if n_k[k] == 0:
            continue
        m = -(-int(n_k[k]) // NCORES)
        fw = m // 128
        r = m % 128
        n_real = int(n_k[k])
        doms_k = dom_order[off : off + n_real]
        off += n_real
        tot = NCORES * m
        core_of = np.arange(tot, dtype=np.int32) % NCORES
        pos_of = np.arange(tot, dtype=np.int32) // NCORES
        # column for position p: in full region if p < fw*128 else ragged
        col_full0 = full_map[k][2] if fw else 0
        p = pos_of[:n_real]
        cols = np.where(
            p < fw * 128,
            col_full0 + p,
            rag_col0 + rag_off.get(k, 0) + (p - fw * 128),
        ).astype(np.int32)
        dom_core[doms_k] = core_of[:n_real]
        dom_col[doms_k] = cols
        cls_meta.append((k, m, n_real, fw, r))

    # pulses sorted by (core, dom column)
    key = dom_core[idx].astype(np.int64) * (1 << 32) + dom_col[idx]
    perm = np.argsort(key, kind="stable")
    core_pulse_counts = np.bincount(dom_core[idx], minlength=NCORES)
    core_splits = np.concatenate([[0], np.cumsum(core_pulse_counts)])

    # ragged window lookup per ragged position
    if nrw:
        rag_kw = np.concatenate(
            [np.full(128, kw, np.int32) for kw, _, _ in rag_win]
        )[: nrw * 128]
        rag_base = np.array([bse for _, bse, _ in rag_win], np.int64)

    # ---- build per-core slot buffers ------------------------------------
    bufs = []
    for c in range(NCORES):
        buf = np.zeros(s_elems, np.float32)
        pc = perm[core_splits[c] : core_splits[c + 1]]
        p_off = 0
        # pass 1: full-window regions, in column order (= ascending k)
        for k, m, n_real, fw, r in cls_meta:
            nreal_c = n_real // NCORES + (1 if c < n_real % NCORES else 0)
            n_full = min(nreal_c, fw * 128)
            if n_full == 0:
                continue
            R_rows = pc[p_off : p_off + n_full * k].reshape(n_full, k)
            p_off += n_full * k
            base = full_map[k][3]
            A = E[R_rows].transpose(0, 2, 1)  # (n, D, k)
            buf[base : base + n_full * D * k] = A.reshape(-1)
        # pass 2: ragged region, in column order (= ascending k)
        for k, m, n_real, fw, r in cls_meta:
            nreal_c = n_real // NCORES + (1 if c < n_real % NCORES else 0)
            n_full = min(nreal_c, fw * 128)
            n_rag = nreal_c - n_full
            if n_rag == 0:
                continue
            R_rows = pc[p_off : p_off + n_rag * k].reshape(n_rag, k)
            p_off += n_rag * k
            rp0 = rag_off[k]
            Arag = E[R_rows].transpose(0, 2, 1)  # (n_rag, D, k)
            i = 0
            while i < n_rag:
                rp = rp0 + i
                j = rp // 128
                kw = int(rag_kw[rp])
                lim = min(n_rag, (j + 1) * 128 - rp0)  # same-window chunk
                chunk = Arag[i:lim]                    # (cn, D, k)
                cn = chunk.shape[0]
                blk = np.empty((cn, D, kw), np.float32)
                blk[:, :, :k] = chunk
                if kw > k:
                    blk[:, :, k:] = chunk[:, :, 0:1]
                bse = int(rag_base[j]) + (rp - j * 128) * D * kw
                buf[bse : bse + cn * D * kw] = blk.reshape(-1)
                i = lim
        bufs.append(buf)

    # ---- weights / tables ----------------------------------------------
    WT = np.ascontiguousarray(W.T)  # (2D, D)
    wts = np.empty((nwt * 128, D), np.float32)
    for jc, (k, fw, col0, base) in enumerate(full_cls):
        blk = WT.copy()
        blk[0:D] *= np.float32(1.0 / k)
        wts[jc * 128 : (jc + 1) * 128] = blk
    wts[len(full_cls) * 128 :] = WT  # unscaled for ragged
    b_col = b.reshape(D, 1)

    rt = None
    if nrw:
        rt = np.zeros((nrw * 128, 2), np.float32)
        rt[:, 0] = 1.0
        kd = np.zeros(nrw * 128, np.int32)
        pos = 0
        for k, r in rag_cls:
            kd[pos : pos + r] = k
            pos += r
        real = kd > 0
        rt[real, 0] = 1.0 / kd[real]
        rt[real, 1] = (rag_kw[real] - kd[real]).astype(np.float32)

    # ---- device ---------------------------------------------------------
    nc = _build_nc(full_cls, rag_win, ndcp, s_elems, nwt, nrw)
    in_maps = []
    for c in range(NCORES):
        m = {"slots": bufs[c], "wts": wts, "b": b_col}
        if nrw:
            m["rt"] = rt
        in_maps.append(m)
    trace = os.environ.get("KERNEL_TRACE", "0") == "1"
    kw_ = {}
    if trace:
        import tempfile
        kw_ = dict(trace=True, tmpdir=tempfile.mkdtemp(prefix="kernel_trace_"))
    res = run_bass_kernel_spmd(nc, in_maps, core_ids=list(range(NCORES)), **kw_)
    last_exec_ns = res.exec_time_ns

    # ---- host-side unpermute -------------------------------------------
    outs = np.stack([res.results[c]["out"] for c in range(NCORES)])  # (8, D, ndcp)
    full = np.empty((nd, D), np.float32)
    real = dom_core >= 0
    full[real] = outs[dom_core[real], :, dom_col[real]]
    if n0:
        full[~real] = b
    return full



# revision 14
# speedup vs baseline: 2.0413x; 2.0413x over previous
"""DOM pooling (segment mean+max over pulses, then linear projection) on 8 trn2 cores.

v2 strategy (bf16 + engine-balanced reduction trees):
  Host: bucket DOMs by exact pulse count k ("classes"); deal DOMs of each class
  round-robin across the 8 cores (identical class structure per core, padded
  with zero doms to an even per-core count m). On each core, consecutive doms
  are PAIRED: SBUF partition p = parity*64 + feat, so one 128-partition column
  holds one slot of two doms. Within a chunk of P dom-pairs the slot buffer is
  slot-major: col = s*P + j  (pair j, slot s), all bf16.

  Device (one NEFF, SPMD on 8 cores), per chunk:
    - DMA load (128, P*k) bf16 (16KB/partition contiguous rows)
    - segment-max: contiguous-halves tensor_tensor tree on DVE (bf16 2x mode)
    - segment-sum: either a DVE add-tree, or folded into the projection on the
      PE via a 0-stride-output accumulating matmul -- chosen per chunk to
      balance DVE vs PE time
    - projection: 128x128 block-diag matmuls (mean scale 1/k folded into the
      per-class sum weights) accumulated in PSUM
    - ACT adds bias during PSUM->SBUF copy (downcast bf16); DMA out.

  Host: scatter per-core (128, N2) outputs back to the full (num_doms, 64).
"""
import sys

import numpy as np

for _p in ("/opt/trn_rl_repo",):
    if _p not in sys.path:
        sys.path.append(_p)

import ml_dtypes

from concourse import bacc
import concourse.mybir as mybir
import concourse.tile as tile
from concourse.bass_utils import run_bass_kernel_spmd

NCORES = 8
D = 64
FP32 = mybir.dt.float32
BF16 = mybir.dt.bfloat16
CHUNK_COLS = 8192   # max slot cols per chunk (16KB/partition bf16)
PMAX = 512          # max dom-pairs per chunk (one PSUM bank of f32)
MM_COLS = 4096      # matmul moving-input column cap

# engine cost constants for load balancing
DVE_NS = 0.55    # ns per 128-lane col, bf16 tensor_tensor in 2x mode
PE_NS = 0.85     # ns per col, bf16 matmul
LDW_NS = 160.0   # per-matmul weight load

last_exec_ns = None  # set when KERNEL_TRACE=1


def _f32_to_bf16_u16(a):
    """Round-to-nearest-even f32 -> bf16 bit pattern (uint16)."""
    u = np.ascontiguousarray(a, dtype=np.float32).view(np.uint32)
    return ((u + 0x7FFF + ((u >> 16) & 1)) >> 16).astype(np.uint16)


def _tree_cols(k, P):
    cols = 0
    w = k
    while w > 1:
        h = w // 2
        cols += h * P
        if w & 1:
            cols += P
        w = h
    return cols


def _plan(counts):
    """Shared class/chunk structure (identical on all cores).

    classes: (k, n_k, m, scol, ocol) ; m per-core doms (even, >= ceil(n_k/8))
    chunks:  (rank, k, c0, P, o0, eng) ; c0 slot-col offset, o0 out-col offset
    """
    kmax = int(counts.max()) if counts.size else 0
    n_k = np.bincount(counts, minlength=kmax + 1)
    classes = []
    scol = 0
    ocol = 0
    for k in range(1, kmax + 1):
        if n_k[k] == 0:
            continue
        m = -(-int(n_k[k]) // NCORES)
        m += m & 1
        classes.append((k, int(n_k[k]), m, scol, ocol))
        scol += (m // 2) * k
        ocol += m // 2
    S, N2 = scol, ocol

    chunks = []
    dve = pe = 0.0
    for rank, (k, nk, m, sc, oc) in enumerate(classes):
        P_k = max(1, min(PMAX, CHUNK_COLS // k))
        pairs = m // 2
        j = 0
        while j < pairs:
            P = min(P_k, pairs - j)
            if k == 1:
                eng = "-"
                pe += LDW_NS + P * PE_NS
            else:
                tcost = _tree_cols(k, P) * DVE_NS
                dve += tcost  # max tree always on DVE
                # D: sum tree also on DVE; P: sum via 0-stride matmul on PE
                d_dve, d_pe = tcost, 2 * LDW_NS + 2 * P * PE_NS
                p_dve, p_pe = 0.0, (k + 1) * (LDW_NS + P * PE_NS)
                if max(dve + d_dve, pe + d_pe) <= max(dve + p_dve, pe + p_pe):
                    eng = "D"
                    dve += d_dve
                    pe += d_pe
                else:
                    eng = "P"
                    pe += p_pe
            chunks.append((rank, k, sc + j * k, P, oc + j, eng))
            j += P
    return classes, chunks, S, N2


def _build_nc(classes, chunks, S, N2):
    nblk = len(classes) + 1  # per-class sum blocks + shared max block
    jmax = len(classes)

    nc = bacc.Bacc(None)
    slots_t = nc.dram_tensor("slots", [128, S], BF16, kind="ExternalInput")
    wts_t = nc.dram_tensor("wts", [nblk * 128, 128], BF16, kind="ExternalInput")
    b_t = nc.dram_tensor("b", [128, 1], FP32, kind="ExternalInput")
    out_t = nc.dram_tensor("out", [128, N2], BF16, kind="ExternalOutput")

    ADD = mybir.AluOpType.add
    MAX = mybir.AluOpType.max

    def emit_tree(eng, dst, src, k, P, op):
        """Reduce k slot-major blocks of P cols: result lands in dst[:, :P].

        Level 0 reads src, writes dst (dst may be src for in-place); later
        levels run in-place on dst. Contiguous operands keep DVE 2x mode.
        """
        w = k
        first = True
        while w > 1:
            h = w // 2
            a = dst if not first else src
            eng.tensor_tensor(
                out=dst[:, : h * P], in0=a[:, : h * P],
                in1=a[:, h * P : 2 * h * P], op=op,
            )
            if w & 1:
                eng.tensor_tensor(
                    out=dst[:, (h - 1) * P : h * P],
                    in0=dst[:, (h - 1) * P : h * P],
                    in1=a[:, 2 * h * P : (2 * h + 1) * P], op=op,
                )
            w = h
            first = False

    with tile.TileContext(nc) as tc:
        with (
            tc.tile_pool(name="const", bufs=1) as constp,
            tc.tile_pool(name="inp", bufs=4) as inp,
            tc.tile_pool(name="tmpp", bufs=4) as tmpp,
            tc.tile_pool(name="outp", bufs=4) as outp,
            tc.tile_pool(name="psp", bufs=4, space="PSUM") as psp,
        ):
            wt_sb = constp.tile([128, nblk * 128], BF16)
            nc.sync.dma_start(
                wt_sb[:].rearrange("p (j e) -> p j e", e=128),
                wts_t[:, :].rearrange("(j p) e -> p j e", p=128),
            )
            b_sb = constp.tile([128, 1], FP32)
            nc.sync.dma_start(b_sb[:], b_t[:])

            for rank, k, c0, P, o0, eng in chunks:
                cols = P * k
                in_t = inp.tile([128, CHUNK_COLS], BF16, tag="in")
                nc.sync.dma_start(in_t[:, :cols], slots_t[:, c0 : c0 + cols])

                ps = psp.tile([128, PMAX], FP32, space="PSUM", tag="ps")
                if k == 1:
                    # sum == max == the slot itself; combined weights block
                    nc.tensor.matmul(
                        ps[:, :P],
                        lhsT=wt_sb[:, rank * 128 : (rank + 1) * 128],
                        rhs=in_t[:, :P],
                        start=True, stop=True,
                    )
                elif eng == "P":
                    # max tree on DVE (non-destructive, into tmp); sum on PE
                    # via one plain + one 0-stride accumulating matmul
                    tmp = tmpp.tile([128, CHUNK_COLS // 2], BF16, tag="tmp")
                    emit_tree(nc.vector, tmp, in_t, k, P, MAX)
                    nc.tensor.matmul(
                        ps[:, :P],
                        lhsT=wt_sb[:, rank * 128 : (rank + 1) * 128],
                        rhs=in_t[:, :P],
                        start=True, stop=False,
                    )
                    for s in range(1, k):
                        nc.tensor.matmul(
                            ps[:, :P],
                            lhsT=wt_sb[:, rank * 128 : (rank + 1) * 128],
                            rhs=in_t[:, s * P : (s + 1) * P],
                            start=False, stop=False,
                        )
                    nc.tensor.matmul(
                        ps[:, :P],
                        lhsT=wt_sb[:, jmax * 128 : (jmax + 1) * 128],
                        rhs=tmp[:, :P],
                        start=False, stop=True,
                    )
                else:
                    # both trees on DVE: sum into tmp, max in-place on in_t
                    tmp = tmpp.tile([128, CHUNK_COLS // 2], BF16, tag="tmp")
                    emit_tree(nc.vector, tmp, in_t, k, P, ADD)
                    emit_tree(nc.vector, in_t, in_t, k, P, MAX)
                    nc.tensor.matmul(
                        ps[:, :P],
                        lhsT=wt_sb[:, rank * 128 : (rank + 1) * 128],
                        rhs=tmp[:, :P],
                        start=True, stop=False,
                    )
                    nc.tensor.matmul(
                        ps[:, :P],
                        lhsT=wt_sb[:, jmax * 128 : (jmax + 1) * 128],
                        rhs=in_t[:, :P],
                        start=False, stop=True,
                    )
                out_sb = outp.tile([128, PMAX], BF16, tag="out")
                nc.scalar.activation(
                    out_sb[:, :P], ps[:, :P],
                    mybir.ActivationFunctionType.Identity, bias=b_sb[:, :1],
                )
                nc.sync.dma_start(out_t[:, o0 : o0 + P], out_sb[:, :P])
    nc.finalize()
    return nc


def kernel(pulse_embeddings, pulse_to_dom_idx, num_doms, proj_w, proj_b):
    global last_exec_ns
    import os

    E = np.ascontiguousarray(np.asarray(pulse_embeddings, dtype=np.float32))
    idx = np.asarray(pulse_to_dom_idx).astype(np.int64)
    nd = int(num_doms)
    W = np.asarray(proj_w, dtype=np.float32)   # (D, 2D)
    b = np.asarray(proj_b, dtype=np.float32)   # (D,)
    NP = E.shape[0]

    counts = np.bincount(idx, minlength=nd)
    classes, chunks, S, N2 = _plan(counts)
    nblk = len(classes) + 1
    jmax = len(classes)

    # ---- dom assignment --------------------------------------------------
    dom_order = np.argsort(counts, kind="stable")
    n0 = int((counts == 0).sum())
    dom_class = np.full(nd, -1, np.int32)
    dom_core = np.zeros(nd, np.int8)
    dom_pos = np.zeros(nd, np.int32)
    off = n0
    for rank, (k, nk, m, sc, oc) in enumerate(classes):
        doms = dom_order[off : off + nk]
        off += nk
        ar = np.arange(nk, dtype=np.int64)
        dom_class[doms] = rank
        dom_core[doms] = ar % NCORES
        dom_pos[doms] = ar // NCORES

    # pulses grouped by (core, class, pos); within a dom original order
    dom_key = (
        (dom_core.astype(np.int64) << 40)
        | (dom_class.astype(np.int64) << 20)
        | dom_pos.astype(np.int64)
    )
    perm = np.argsort(dom_key[idx], kind="stable").astype(np.int32)

    # pulse count per (core, class): n_c * k
    core_cls_pulses = np.zeros((NCORES, len(classes)), np.int64)
    for rank, (k, nk, m, sc, oc) in enumerate(classes):
        n_c = nk // NCORES + (np.arange(NCORES) < nk % NCORES)
        core_cls_pulses[:, rank] = n_c * k
    core_off = np.concatenate([[0], np.cumsum(core_cls_pulses.sum(axis=1))])

    # ---- slot buffers ----------------------------------------------------
    Eb = _f32_to_bf16_u16(E)                      # (NP, 64) uint16
    E2b = np.vstack([Eb, np.zeros((1, D), np.uint16)])
    Z = NP

    bufs = []
    for c in range(NCORES):
        blocks = []
        p_off = int(core_off[c])
        for rank, (k, nk, m, sc, oc) in enumerate(classes):
            n_c = nk // NCORES + (1 if c < nk % NCORES else 0)
            R = np.full((m, k), Z, np.int32)
            if n_c:
                R[:n_c] = perm[p_off : p_off + n_c * k].reshape(n_c, k)
                p_off += n_c * k
            R2 = R.reshape(m // 2, 2, k)
            P_k = max(1, min(PMAX, CHUNK_COLS // k))
            j = 0
            while j < m // 2:
                P = min(P_k, m // 2 - j)
                blk = R2[j : j + P]                       # (P, 2, k)
                blocks.append(blk.transpose(1, 2, 0).reshape(2, k * P))
                j += P
        ridx = np.concatenate(blocks, axis=1)             # (2, S)
        g = E2b[ridx]                                     # (2, S, 64) uint16
        buf = np.ascontiguousarray(g.transpose(0, 2, 1)).reshape(128, S)
        bufs.append(buf.view(ml_dtypes.bfloat16))

    # ---- weights / bias --------------------------------------------------
    Wsum = W[:, :D]
    Wmax = W[:, D:]

    def blkdiag(M):
        Z2 = np.zeros((128, 128), np.float32)
        Z2[:D, :D] = M
        Z2[D:, D:] = M
        return Z2

    wblocks = []
    for rank, (k, nk, m, sc, oc) in enumerate(classes):
        if k == 1:
            wblocks.append(blkdiag((Wsum + Wmax).T))
        else:
            wblocks.append(blkdiag(Wsum.T / np.float32(k)))
    wblocks.append(blkdiag(Wmax.T))
    wts = _f32_to_bf16_u16(np.concatenate(wblocks, axis=0)).view(ml_dtypes.bfloat16)
    b128 = np.concatenate([b, b]).reshape(128, 1).astype(np.float32)

    # ---- device ----------------------------------------------------------
    nc = _build_nc(classes, chunks, S, N2)
    in_maps = [{"slots": bufs[c], "wts": wts, "b": b128} for c in range(NCORES)]
    trace = os.environ.get("KERNEL_TRACE", "0") == "1"
    kw_ = {}
    if trace:
        import tempfile
        kw_ = dict(trace=True, tmpdir=tempfile.mkdtemp(prefix="kernel_trace_"))
    res = run_bass_kernel_spmd(nc, in_maps, core_ids=list(range(NCORES)), **kw_)
    last_exec_ns = res.exec_time_ns

    # ---- scatter back ----------------------------------------------------
    outs = np.stack(
        [np.asarray(res.results[c]["out"], dtype=np.float32) for c in range(NCORES)]
    )                                                     # (8, 128, N2)
    outs = outs.reshape(NCORES, 2, D, N2)
    cls_ocol = np.array([oc for (k, nk, m, sc, oc) in classes], np.int64)
    real = counts > 0
    d_core = dom_core[real].astype(np.int64)
    d_ocol = cls_ocol[dom_class[real]] + dom_pos[real] // 2
    d_par = dom_pos[real] % 2
    full = np.empty((nd, D), np.float32)
    full[real] = outs[d_core, d_par, :, d_ocol]
    full[~real] = b
    return full


# revision 36
# speedup vs baseline: 2.4749x; 1.2124x over previous
"""DOM pooling (segment mean+max over pulses, then linear projection) on 8 trn2 cores.

Strategy (bf16 + engine-balanced reductions):
  Host: bucket DOMs by exact pulse count k ("classes"); deal DOMs of each class
  round-robin across the 8 cores (identical class structure per core, padded
  with zero doms to an even per-core count m). On each core, consecutive doms
  are PAIRED: SBUF partition p = parity*64 + feat, so one 128-partition column
  holds one slot of two doms. Within a chunk of P dom-pairs the slot buffer is
  slot-major: col = s*P + j  (pair j, slot s), all bf16.

  Device (one NEFF, SPMD on 8 cores), per chunk:
    - DMA load (128, P*k) bf16 (16KB/partition contiguous rows)
    - segment-max: contiguous-halves tensor_tensor tree on DVE (bf16 2x mode)
    - segment-sum: either a DVE add-tree, or folded into the projection on the
      PE via per-slot PSUM-accumulating matmuls -- chosen per chunk to balance
      DVE vs PE time
    - projection: 128x128 block-diag matmuls (mean scale 1/k folded into the
      per-class sum weights) accumulated in PSUM
    - ACT adds bias during PSUM->SBUF copy (downcast bf16); DMA out.

  Host: scatter per-core (128, N2) outputs back to the full (num_doms, 64).
"""
import sys

import numpy as np

for _p in ("/opt/trn_rl_repo",):
    if _p not in sys.path:
        sys.path.append(_p)

import ml_dtypes

from concourse import bacc
import concourse.mybir as mybir
import concourse.tile as tile
from concourse.bass_utils import run_bass_kernel_spmd

NCORES = 8
D = 64
FP32 = mybir.dt.float32
BF16 = mybir.dt.bfloat16
CHUNK_COLS = 8192   # max slot cols per chunk (16KB/partition bf16)
PMAX = 512          # max dom-pairs per chunk (one PSUM bank of f32)

# engine cost constants for load balancing (calibrated from HW traces)
DVE_NS = 0.68    # ns per 128-lane col, bf16 tensor_tensor in 2x mode
PE_NS = 0.40     # ns per col, bf16 matmul
LDW_NS = 50.0    # per-matmul weight load

last_exec_ns = None  # set when KERNEL_TRACE=1


def _f32_to_bf16_u16(a):
    """Round-to-nearest-even f32 -> bf16 bit pattern (uint16)."""
    u = np.ascontiguousarray(a, dtype=np.float32).view(np.uint32)
    return ((u + 0x7FFF + ((u >> 16) & 1)) >> 16).astype(np.uint16)


def _tree_cols(k, P):
    cols = 0
    w = k
    while w > 1:
        h = w // 2
        cols += h * P
        if w & 1:
            cols += P
        w = h
    return cols


def _plan(counts):
    """Shared class/chunk structure (identical on all cores).

    classes: (k, n_k, m, scol, ocol) ; m per-core doms (even, >= ceil(n_k/8))
    chunks:  (rank, k, c0, P, o0, eng) ; c0 slot-col offset, o0 out-col offset
    """
    kmax = int(counts.max()) if counts.size else 0
    n_k = np.bincount(counts, minlength=kmax + 1)
    classes = []
    scol = 0
    ocol = 0
    for k in range(1, kmax + 1):
        if n_k[k] == 0:
            continue
        m = -(-int(n_k[k]) // NCORES)
        m += m & 1
        classes.append((k, int(n_k[k]), m, scol, ocol))
        scol += (m // 2) * k
        ocol += m // 2
    S, N2 = scol, ocol

    raw = []
    for rank, (k, nk, m, sc, oc) in enumerate(classes):
        P_k = max(1, min(PMAX, CHUNK_COLS // k))
        pairs = m // 2
        j = 0
        while j < pairs:
            P = min(P_k, pairs - j)
            raw.append((rank, k, sc + j * k, P, oc + j))
            j += P
    # emission order: a few small chunks first (fast pipeline fill), then
    # large ones, smallest last (short drain tail). Slot/out offsets are
    # absolute, so processing order is free.
    asc = sorted(range(len(raw)), key=lambda i: raw[i][1] * raw[i][3])
    head, tail, mid = asc[:4], asc[4:12][::-1], asc[12:][::-1]
    raw = [raw[i] for i in head + mid + tail]
    # engine assignment (greedy balance) in emission order
    chunks = []
    dve = pe = 0.0
    for rank, k, c0, P, o0 in raw:
        if k == 1:
            eng = "-"
            pe += LDW_NS + P * PE_NS
        else:
            tcost = _tree_cols(k, P) * DVE_NS
            dve += tcost  # max tree always on DVE
            # D: sum tree also on DVE; P: sum via accumulating matmuls on PE
            d_dve, d_pe = tcost, 2 * LDW_NS + 2 * P * PE_NS
            p_dve, p_pe = 0.0, (k + 1) * (LDW_NS + P * PE_NS)
            if max(dve + d_dve, pe + d_pe) <= max(dve + p_dve, pe + p_pe):
                eng = "D"
                dve += d_dve
                pe += d_pe
            else:
                eng = "P"
                pe += p_pe
        chunks.append((rank, k, c0, P, o0, eng))
    return classes, chunks, S, N2


def _build_nc(classes, chunks, S, N2):
    nblk = len(classes) + 1  # per-class sum blocks + shared max block
    jmax = len(classes)

    nc = bacc.Bacc(None)
    slots_t = nc.dram_tensor("slots", [128, S], BF16, kind="ExternalInput")
    # weights pre-transposed on host to the SBUF layout (one clean DMA)
    wts_t = nc.dram_tensor("wts", [128, nblk * 128], BF16, kind="ExternalInput")
    b_t = nc.dram_tensor("b", [128, 1], FP32, kind="ExternalInput")
    out_t = nc.dram_tensor("out", [128, N2], BF16, kind="ExternalOutput")

    ADD = mybir.AluOpType.add
    MAX = mybir.AluOpType.max

    def emit_tree(eng, dst, src, k, P, op):
        """Reduce k slot-major blocks of P cols: result lands in dst[:, :P].

        Level 0 reads src, writes dst (dst may be src for in-place); later
        levels run in-place on dst. Contiguous operands keep DVE 2x mode.
        """
        w = k
        first = True
        while w > 1:
            h = w // 2
            a = dst if not first else src
            eng.tensor_tensor(
                out=dst[:, : h * P], in0=a[:, : h * P],
                in1=a[:, h * P : 2 * h * P], op=op,
            )
            if w & 1:
                eng.tensor_tensor(
                    out=dst[:, (h - 1) * P : h * P],
                    in0=dst[:, (h - 1) * P : h * P],
                    in1=a[:, 2 * h * P : (2 * h + 1) * P], op=op,
                )
            w = h
            first = False

    with tile.TileContext(nc) as tc:
        with (
            tc.tile_pool(name="const", bufs=1) as constp,
            tc.tile_pool(name="inp", bufs=6) as inp,
            tc.tile_pool(name="tmpp", bufs=4) as tmpp,
            tc.tile_pool(name="outp", bufs=4) as outp,
            tc.tile_pool(name="psp", bufs=4, space="PSUM") as psp,
        ):
            # weights/bias on the ACT ring so they overlap the first chunk
            # loads on the SP ring (matmuls are the only consumers)
            wt_sb = constp.tile([128, nblk * 128], BF16)
            nc.scalar.dma_start(wt_sb[:], wts_t[:, :])
            b_sb = constp.tile([128, 1], FP32)
            nc.scalar.dma_start(b_sb[:], b_t[:])

            for rank, k, c0, P, o0, eng in chunks:
                cols = P * k
                in_t = inp.tile([128, CHUNK_COLS], BF16, tag="in")
                # split the load across both HWDGE rings (SP + ACT)
                h2 = (cols // 2) if cols >= 64 else 0
                if h2:
                    nc.sync.dma_start(in_t[:, :h2], slots_t[:, c0 : c0 + h2])
                    nc.scalar.dma_start(
                        in_t[:, h2:cols], slots_t[:, c0 + h2 : c0 + cols]
                    )
                else:
                    nc.sync.dma_start(in_t[:, :cols], slots_t[:, c0 : c0 + cols])

                ps = psp.tile([128, PMAX], FP32, space="PSUM", tag="ps")
                if k == 1:
                    # sum == max == the slot itself; combined weights block
                    nc.tensor.matmul(
                        ps[:, :P],
                        lhsT=wt_sb[:, rank * 128 : (rank + 1) * 128],
                        rhs=in_t[:, :P],
                        start=True, stop=True,
                    )
                elif eng == "P":
                    # max tree on DVE (non-destructive, into tmp); sum on PE
                    # via per-slot PSUM-accumulating matmuls
                    tmp = tmpp.tile([128, CHUNK_COLS // 2], BF16, tag="tmp")
                    emit_tree(nc.vector, tmp, in_t, k, P, MAX)
                    nc.tensor.matmul(
                        ps[:, :P],
                        lhsT=wt_sb[:, rank * 128 : (rank + 1) * 128],
                        rhs=in_t[:, :P],
                        start=True, stop=False,
                    )
                    for s in range(1, k):
                        nc.tensor.matmul(
                            ps[:, :P],
                            lhsT=wt_sb[:, rank * 128 : (rank + 1) * 128],
                            rhs=in_t[:, s * P : (s + 1) * P],
                            start=False, stop=False,
                        )
                    nc.tensor.matmul(
                        ps[:, :P],
                        lhsT=wt_sb[:, jmax * 128 : (jmax + 1) * 128],
                        rhs=tmp[:, :P],
                        start=False, stop=True,
                    )
                else:
                    # both trees on DVE: sum into tmp, max in-place on in_t
                    tmp = tmpp.tile([128, CHUNK_COLS // 2], BF16, tag="tmp")
                    emit_tree(nc.vector, tmp, in_t, k, P, ADD)
                    emit_tree(nc.vector, in_t, in_t, k, P, MAX)
                    nc.tensor.matmul(
                        ps[:, :P],
                        lhsT=wt_sb[:, rank * 128 : (rank + 1) * 128],
                        rhs=tmp[:, :P],
                        start=True, stop=False,
                    )
                    nc.tensor.matmul(
                        ps[:, :P],
                        lhsT=wt_sb[:, jmax * 128 : (jmax + 1) * 128],
                        rhs=in_t[:, :P],
                        start=False, stop=True,
                    )
                out_sb = outp.tile([128, PMAX], BF16, tag="out")
                nc.scalar.activation(
                    out_sb[:, :P], ps[:, :P],
                    mybir.ActivationFunctionType.Identity, bias=b_sb[:, :1],
                )
                nc.sync.dma_start(out_t[:, o0 : o0 + P], out_sb[:, :P])
    nc.finalize()
    return nc


def kernel(pulse_embeddings, pulse_to_dom_idx, num_doms, proj_w, proj_b):
    global last_exec_ns
    import os

    E = np.ascontiguousarray(np.asarray(pulse_embeddings, dtype=np.float32))
    idx = np.asarray(pulse_to_dom_idx).astype(np.int64)
    nd = int(num_doms)
    W = np.asarray(proj_w, dtype=np.float32)   # (D, 2D)
    b = np.asarray(proj_b, dtype=np.float32)   # (D,)
    NP = E.shape[0]

    counts = np.bincount(idx, minlength=nd)
    classes, chunks, S, N2 = _plan(counts)

    # ---- dom assignment --------------------------------------------------
    dom_order = np.argsort(counts, kind="stable")
    n0 = int((counts == 0).sum())
    dom_class = np.full(nd, -1, np.int32)
    dom_core = np.zeros(nd, np.int8)
    dom_pos = np.zeros(nd, np.int32)
    off = n0
    for rank, (k, nk, m, sc, oc) in enumerate(classes):
        doms = dom_order[off : off + nk]
        off += nk
        ar = np.arange(nk, dtype=np.int64)
        dom_class[doms] = rank
        dom_core[doms] = ar % NCORES
        dom_pos[doms] = ar // NCORES

    # pulses grouped by (core, class, pos); within a dom original order
    dom_key = (
        (dom_core.astype(np.int64) << 40)
        | (dom_class.astype(np.int64) << 20)
        | dom_pos.astype(np.int64)
    )
    perm = np.argsort(dom_key[idx], kind="stable").astype(np.int32)

    # pulse count per (core, class): n_c * k
    core_cls_pulses = np.zeros((NCORES, len(classes)), np.int64)
    for rank, (k, nk, m, sc, oc) in enumerate(classes):
        n_c = nk // NCORES + (np.arange(NCORES) < nk % NCORES)
        core_cls_pulses[:, rank] = n_c * k
    core_off = np.concatenate([[0], np.cumsum(core_cls_pulses.sum(axis=1))])

    # ---- slot buffers ----------------------------------------------------
    Eb = _f32_to_bf16_u16(E)                      # (NP, 64) uint16
    E2b = np.vstack([Eb, np.zeros((1, D), np.uint16)])
    Z = NP

    bufs = []
    for c in range(NCORES):
        blocks = []
        p_off = int(core_off[c])
        for rank, (k, nk, m, sc, oc) in enumerate(classes):
            n_c = nk // NCORES + (1 if c < nk % NCORES else 0)
            R = np.full((m, k), Z, np.int32)
            if n_c:
                R[:n_c] = perm[p_off : p_off + n_c * k].reshape(n_c, k)
                p_off += n_c * k
            R2 = R.reshape(m // 2, 2, k)
            P_k = max(1, min(PMAX, CHUNK_COLS // k))
            j = 0
            while j < m // 2:
                P = min(P_k, m // 2 - j)
                blk = R2[j : j + P]                       # (P, 2, k)
                blocks.append(blk.transpose(1, 2, 0).reshape(2, k * P))
                j += P
        ridx = np.concatenate(blocks, axis=1)             # (2, S)
        g = E2b[ridx]                                     # (2, S, 64) uint16
        buf = np.ascontiguousarray(g.transpose(0, 2, 1)).reshape(128, S)
        bufs.append(buf.view(ml_dtypes.bfloat16))

    # ---- weights / bias --------------------------------------------------
    Wsum = W[:, :D]
    Wmax = W[:, D:]

    def blkdiag(M):
        Z2 = np.zeros((128, 128), np.float32)
        Z2[:D, :D] = M
        Z2[D:, D:] = M
        return Z2

    wblocks = []
    for rank, (k, nk, m, sc, oc) in enumerate(classes):
        if k == 1:
            wblocks.append(blkdiag((Wsum + Wmax).T))
        else:
            wblocks.append(blkdiag(Wsum.T / np.float32(k)))
    wblocks.append(blkdiag(Wmax.T))
    # (nblk*128, 128) -> SBUF layout (128, nblk*128): partition p, col j*128+e
    wcat = np.concatenate(wblocks, axis=0).reshape(-1, 128, 128)
    wcat = np.ascontiguousarray(wcat.transpose(1, 0, 2)).reshape(128, -1)
    wts = _f32_to_bf16_u16(wcat).view(ml_dtypes.bfloat16)
    b128 = np.concatenate([b, b]).reshape(128, 1).astype(np.float32)

    # ---- device ----------------------------------------------------------
    nc = _build_nc(classes, chunks, S, N2)
    in_maps = [{"slots": bufs[c], "wts": wts, "b": b128} for c in range(NCORES)]
    trace = os.environ.get("KERNEL_TRACE", "0") == "1"
    kw_ = {}
    if trace:
        import tempfile
        kw_ = dict(trace=True, tmpdir=tempfile.mkdtemp(prefix="kernel_trace_"))
    res = run_bass_kernel_spmd(nc, in_maps, core_ids=list(range(NCORES)), **kw_)
    last_exec_ns = res.exec_time_ns

    # ---- scatter back ----------------------------------------------------
    outs = np.stack(
        [np.asarray(res.results[c]["out"], dtype=np.float32) for c in range(NCORES)]
    )                                                     # (8, 128, N2)
    outs = outs.reshape(NCORES, 2, D, N2)
    cls_ocol = np.array([oc for (k, nk, m, sc, oc) in classes], np.int64)
    real = counts > 0
    d_core = dom_core[real].astype(np.int64)
    d_ocol = cls_ocol[dom_class[real]] + dom_pos[real] // 2
    d_par = dom_pos[real] % 2
    full = np.empty((nd, D), np.float32)
    full[real] = outs[d_core, d_par, :, d_ocol]
    full[~real] = b
    return full
